# revision 2
# baseline (speedup 1.0000x reference)
"""Trainium2 Bass kernel for a 2-layer BiLSTM regressor (B=256, T=512, F=32,
H=100, relu candidate/output activations, sigmoid gates) + TimeDistributed
Dense(8, relu) head.

Strategy: data-parallel over batch across 8 NeuronCores (32 sequences/core).
Sequences live on-chip transposed as [H, T*B] bf16 tensors (column = t*B+b).

The two directions of a layer run as two PHASE-SHIFTED streams (not lockstep):
each direction has its own per-window PSUM tensor with gate regions laid out
[i|f|o|g] (weights column-reordered on host), its own sigmoid ACT, DVE chain
and Pool op.  While direction F is in its ACT hop, direction B runs its DVE
chain, halving the effective latency of the sequential recurrence:

 - Per wall-slot per direction: 4 recurrent matmuls (N=32) accumulate Wr^T
   h_prev onto the projections; ONE Sigmoid ACT covers (i,f,o) via a strided
   PSUM read; DVE computes u = relu(g)*sig_i straight from PSUM g via
   scalar_tensor_tensor (no separate relu instruction), the Pool engine
   computes sig_f*c in parallel, DVE finishes c = sig_f*c + u and
   h = relu(c)*sig_o.
 - Next-window input projections are emitted at the head of each slot so they
   fill PE idle gaps without head-of-line blocking the recurrent matmuls.
 - h is written to small bf16 staging tiles feeding the next slot's matmuls;
   SBUF->SBUF DMAs flush them to the big sequence buffers off the critical
   path.  Biases ride the projections via a constant-ones row; the Dense head
   is a final matmul pass.  Host does the cheap input/output transposes.
"""

import numpy as np
import ml_dtypes
from contextlib import ExitStack

H = 100          # LSTM units per direction
F = 32           # input features
NT = 8           # dense head outputs
T_FULL = 512
B_FULL = 256
N_CORES = 8
B_LOC = B_FULL // N_CORES   # 32
W = 8            # timesteps per PSUM window
# device gate region order in the 4H axis: i, f, o, g (host reorders weights)
G_I, G_F, G_O, G_G = 0, 1, 2, 3

_BUILD_CACHE = {}
LAST_RESULTS = None  # BassKernelResults of the most recent run (for test.py)


def build_nc(T=T_FULL, B=B_LOC):
    """Build (and bacc-compile) the Bass program for one core."""
    key = (T, B)
    if key in _BUILD_CACHE:
        return _BUILD_CACHE[key]

    import concourse.bacc as bacc
    import concourse.tile as tile
    from concourse import mybir

    fp32 = mybir.dt.float32
    bf16 = mybir.dt.bfloat16
    MAX = mybir.AluOpType.max
    MULT = mybir.AluOpType.mult
    SIG = mybir.ActivationFunctionType.Sigmoid
    RELU = mybir.ActivationFunctionType.Relu

    assert T % W == 0
    TB = T * B
    TB1 = (T + 1) * B
    WB = W * B          # region size (256 cols)

    nc = bacc.Bacc("TRN2", target_bir_lowering=False, debug=False)

    # ---- DRAM I/O ----
    d_x = nc.dram_tensor("x_t", [F + 1, TB], bf16, kind="ExternalInput")
    d_ones = nc.dram_tensor("ones_row", [1, TB1], bf16, kind="ExternalInput")
    dw = {}
    for name, shape in [
        ("wk1f", [F + 1, 4 * H]), ("wr1f", [H, 4 * H]),
        ("wk1b", [F + 1, 4 * H]), ("wr1b", [H, 4 * H]),
        ("wk2f_f", [H + 1, 4 * H]), ("wk2f_b", [H, 4 * H]), ("wr2f", [H, 4 * H]),
        ("wk2b_f", [H + 1, 4 * H]), ("wk2b_b", [H, 4 * H]), ("wr2b", [H, 4 * H]),
        ("wdf", [H + 1, NT]), ("wdb", [H, NT]),
    ]:
        dw[name] = nc.dram_tensor(name, shape, bf16, kind="ExternalInput")
    d_y = nc.dram_tensor("y_t", [NT, TB], fp32, kind="ExternalOutput")

    with tile.TileContext(nc) as tc, ExitStack() as ctx:
        persist = ctx.enter_context(tc.tile_pool(name="persist", bufs=1))
        psum = ctx.enter_context(tc.tile_pool(name="psum", bufs=4, space="PSUM"))
        small = ctx.enter_context(tc.tile_pool(name="small", bufs=4))
        outp = ctx.enter_context(tc.tile_pool(name="outp", bufs=4))

        # ---- persistent SBUF tensors ----
        sb_x = persist.tile([F + 1, TB], bf16, name="sb_x")
        # fwd sequences keep h(phys t) at block t+1 (block 0 = zeros);
        # bwd sequences keep h(phys t) at block t (block T = zeros).
        sb_h1f = persist.tile([H + 1, TB1], bf16, name="sb_h1f")
        sb_h1b = persist.tile([H, TB1], bf16, name="sb_h1b")
        sb_h2f = persist.tile([H + 1, TB1], bf16, name="sb_h2f")
        sb_h2b = persist.tile([H, TB1], bf16, name="sb_h2b")
        sbw = {}
        for name, d in dw.items():
            sbw[name] = persist.tile(list(d.shape), bf16, name="sb_" + name)
            nc.sync.dma_start(out=sbw[name], in_=d.ap())
        nc.sync.dma_start(out=sb_x, in_=d_x.ap())
        # ones rows (bias folding) via DMA (vector memset over 16K cols is slow)
        nc.sync.dma_start(out=sb_h1f[H:H + 1, :], in_=d_ones.ap())
        nc.sync.dma_start(out=sb_h2f[H:H + 1, :], in_=d_ones.ap())

        # zero-filled "previous h" staging used at each layer's first slot
        z_stag = persist.tile([H, B], bf16, name="z_stag")
        nc.vector.memset(z_stag, 0.0)
        cst = {}
        for lname in ("c1f", "c1b", "c2f", "c2b"):
            cst[lname] = persist.tile([H, B], fp32, name=lname)
            nc.vector.memset(cst[lname], 0.0)

        def xrhs(src, K, coff, t_lo, reverse):
            """Window rhs [K, W*B]; block order reversed for bwd."""
            sl = src[0:K, coff + t_lo * B: coff + (t_lo + W) * B]
            if not reverse:
                return sl
            return sl.rearrange("p (w b) -> p w b", b=B)[:, ::-1, :]

        def bilstm(xsrc_f, xsrc_b, wr_f, wr_b, out_f, out_b, c_f, c_b):
            """One bidirectional layer; directions run as phase-shifted
            streams sharing no instructions."""
            nwin = T // W
            wr = (wr_f, wr_b)
            cdir = (c_f, c_b)
            xsrc = (xsrc_f, xsrc_b)
            stag_prev = [z_stag[:, :], z_stag[:, :]]

            def make_win():
                tiles = {}

                def get_tile(d):
                    if d not in tiles:
                        tiles[d] = psum.tile([H, 4 * WB], fp32, tag="gates",
                                             name=f"ps{d}")
                    return tiles[d]
                return get_tile

            def xproj_thunks(w, get_tile):
                t0f = w * W
                thb = T - 1 - w * W
                thunks = []
                started = set()
                for di in (0, 1):
                    t_lo = t0f if di == 0 else thb - W + 1
                    for g in range(4):
                        off = g * WB
                        for wk, src, K_, coff in xsrc[di]:
                            key = (di, off // 512)
                            first = key not in started
                            started.add(key)

                            def thunk(di=di, g=g, off=off, t_lo=t_lo, wk=wk,
                                      src=src, K_=K_, coff=coff, first=first):
                                pt = get_tile(di)
                                nc.tensor.matmul(
                                    out=pt[:, off:off + WB],
                                    lhsT=wk[:, g * H:(g + 1) * H],
                                    rhs=xrhs(src, K_, coff, t_lo, di == 1),
                                    start=first, stop=False,
                                    skip_group_check=True)
                            thunks.append(thunk)
                return thunks

            get_tile = make_win()
            for th in xproj_thunks(0, get_tile):
                th()
            for w in range(nwin):
                t0f = w * W                    # fwd slot s -> phys t0f + s
                thb = T - 1 - w * W            # bwd slot s -> phys thb - s
                if w + 1 < nwin:
                    next_get = make_win()
                    next_thunks = xproj_thunks(w + 1, next_get)
                else:
                    next_get, next_thunks = None, []
                # spread next window's projections over this window's slots;
                # they are dependency-free so the in-order PE queue drains
                # them during recurrence gaps when emitted at slot head
                per_slot = (len(next_thunks) + W - 1) // W
                wstag = (small.tile([H, WB], bf16, tag="wstf", bufs=2,
                                    name="wstf"),
                         small.tile([H, WB], bf16, tag="wstb", bufs=2,
                                    name="wstb"))
                for k in range(W):
                    for th in next_thunks[k * per_slot:(k + 1) * per_slot]:
                        th()
                    ksl = slice(k * B, (k + 1) * B)
                    for di in (0, 1):
                        pt = get_tile(di)
                        # recurrent matmuls i,f,o then g (sigmoid fires on
                        # i,f,o; g is only needed later by the DVE u-op)
                        for g in range(4):
                            nc.tensor.matmul(
                                out=pt[:, g * WB + k * B: g * WB + (k + 1) * B],
                                lhsT=wr[di][:, g * H:(g + 1) * H],
                                rhs=stag_prev[di],
                                start=False, stop=True, skip_group_check=True)
                        ptv = pt.rearrange("p (r n) -> p r n", r=4)
                        sig = small.tile([H, 3, B], fp32, tag=f"sig{di}",
                                         bufs=3, name=f"sig{di}")
                        nc.scalar.activation(sig, ptv[:, 0:3, ksl], SIG)
                        # u = relu(g) * sig_i, straight from PSUM
                        u = small.tile([H, B], fp32, tag=f"u{di}", bufs=2,
                                       name=f"u{di}")
                        nc.vector.scalar_tensor_tensor(
                            out=u, in0=ptv[:, 3, ksl], scalar=0.0,
                            in1=sig[:, 0, :], op0=MAX, op1=MULT)
                        # sig_f * c on the Pool engine, in parallel with u
                        c2 = small.tile([H, B], fp32, tag=f"c2{di}", bufs=2,
                                        name=f"c2{di}")
                        nc.gpsimd.tensor_mul(c2, sig[:, 1, :], cdir[di])
                        nc.vector.tensor_add(cdir[di], c2, u)
                        # h = relu(c) * sig_o -> window staging (bf16)
                        nc.vector.scalar_tensor_tensor(
                            out=wstag[di][:, ksl], in0=cdir[di], scalar=0.0,
                            in1=sig[:, 2, :], op0=MAX, op1=MULT)
                        stag_prev[di] = wstag[di][:, ksl]
                # flush the window's h values to the sequence buffers:
                # fwd slots are phys-ascending (contiguous); bwd slots are
                # phys-descending, so reverse the block order on the read side
                nc.sync.dma_start(
                    out=out_f[0:H, (t0f + 1) * B:(t0f + 1 + W) * B],
                    in_=wstag[0])
                wstag_b = wstag[1].rearrange("p (w b) -> p w b", b=B)
                nc.sync.dma_start(
                    out=out_b[0:H, (thb - W + 1) * B:(thb + 1) * B],
                    in_=wstag_b[:, ::-1, :])
                if next_get is not None:
                    get_tile = next_get

        # layer 1: input = x (K = F+1 with bias row)
        bilstm(
            xsrc_f=[(sbw["wk1f"], sb_x, F + 1, 0)],
            xsrc_b=[(sbw["wk1b"], sb_x, F + 1, 0)],
            wr_f=sbw["wr1f"], wr_b=sbw["wr1b"],
            out_f=sb_h1f, out_b=sb_h1b, c_f=cst["c1f"], c_b=cst["c1b"])
        # layer 2: input = [h1f (blocks +1, ones row) ; h1b]
        bilstm(
            xsrc_f=[(sbw["wk2f_f"], sb_h1f, H + 1, B), (sbw["wk2f_b"], sb_h1b, H, 0)],
            xsrc_b=[(sbw["wk2b_f"], sb_h1f, H + 1, B), (sbw["wk2b_b"], sb_h1b, H, 0)],
            wr_f=sbw["wr2f"], wr_b=sbw["wr2b"],
            out_f=sb_h2f, out_b=sb_h2b, c_f=cst["c2f"], c_b=cst["c2b"])

        # dense head: y = relu(Wd^T [h2f;h2b] + bd) over 512-col chunks
        CH = 512
        nch = (TB + CH - 1) // CH
        for ci in range(nch):
            c0 = ci * CH
            n = min(CH, TB - c0)
            ps = psum.tile([NT, CH], fp32, tag="gates", name="ps_y")
            nc.tensor.matmul(out=ps[:, 0:n], lhsT=sbw["wdf"],
                             rhs=sb_h2f[0:H + 1, B + c0: B + c0 + n],
                             start=True, stop=False, skip_group_check=True)
            nc.tensor.matmul(out=ps[:, 0:n], lhsT=sbw["wdb"],
                             rhs=sb_h2b[0:H, c0: c0 + n],
                             start=False, stop=True, skip_group_check=True)
            st = outp.tile([NT, CH], fp32, tag="stage", name="st_y")
            if ci % 2 == 0:
                nc.scalar.activation(st[:, 0:n], ps[:, 0:n], RELU)
            else:
                nc.vector.tensor_single_scalar(st[:, 0:n], ps[:, 0:n], 0.0, MAX)
            nc.sync.dma_start(out=d_y.ap()[:, c0:c0 + n], in_=st[:, 0:n])

    nc.compile()
    _BUILD_CACHE[key] = nc
    return nc


# ---------------------------------------------------------------------------
# host side
# ---------------------------------------------------------------------------

def _bf16(a):
    return np.asarray(a, np.float32).astype(ml_dtypes.bfloat16)


# device gate order is i,f,o,g; keras order is i,f,g,o
_PERM = np.concatenate([np.arange(0, 2 * H), np.arange(3 * H, 4 * H),
                        np.arange(2 * H, 3 * H)])


def _gr(w):
    """Reorder keras gate columns i,f,g,o -> device order i,f,o,g."""
    return np.asarray(w, np.float32)[..., _PERM]


def prepare_weight_maps(Wk1f, Wr1f, b1f, Wk1b, Wr1b, b1b,
                        Wk2f, Wr2f, b2f, Wk2b, Wr2b, b2b, Wd, bd):
    def aug(w, b):
        return np.vstack([np.asarray(w, np.float32),
                          np.asarray(b, np.float32)[None, :]])
    m = {
        "wk1f": aug(_gr(Wk1f), _gr(b1f)), "wr1f": _gr(Wr1f),
        "wk1b": aug(_gr(Wk1b), _gr(b1b)), "wr1b": _gr(Wr1b),
        "wk2f_f": aug(_gr(Wk2f)[:H], _gr(b2f)), "wk2f_b": _gr(Wk2f)[H:],
        "wr2f": _gr(Wr2f),
        "wk2b_f": aug(_gr(Wk2b)[:H], _gr(b2b)), "wk2b_b": _gr(Wk2b)[H:],
        "wr2b": _gr(Wr2b),
        "wdf": aug(Wd[:H], bd), "wdb": Wd[H:],
    }
    return {k: np.ascontiguousarray(_bf16(v)) for k, v in m.items()}


def make_in_maps(x, weights, T, B):
    """x: [Btot, T, F] fp32 -> list of per-core input dicts."""
    n_cores = x.shape[0] // B
    ones = np.ones((1, (T + 1) * B), ml_dtypes.bfloat16)
    in_maps = []
    for c in range(n_cores):
        xc = np.asarray(x[c * B:(c + 1) * B], np.float32)      # [B, T, F]
        xt = xc.transpose(2, 1, 0).reshape(F, T * B)           # [F, T*B] t-major
        x_aug = np.vstack([xt, np.ones((1, T * B), np.float32)])
        in_maps.append({"x_t": np.ascontiguousarray(_bf16(x_aug)),
                        "ones_row": ones, **weights})
    return in_maps


def kernel(x, Wk1f, Wr1f, b1f, Wk1b, Wr1b, b1b,
           Wk2f, Wr2f, b2f, Wk2b, Wr2b, b2b, Wd, bd,
           trace=False):
    global LAST_RESULTS
    from concourse.bass_utils import run_bass_kernel_spmd

    Btot, T, _ = x.shape
    B = Btot // N_CORES
    nc = build_nc(T, B)
    weights = prepare_weight_maps(Wk1f, Wr1f, b1f, Wk1b, Wr1b, b1b,
                                  Wk2f, Wr2f, b2f, Wk2b, Wr2b, b2b, Wd, bd)
    in_maps = make_in_maps(x, weights, T, B)
    res = run_bass_kernel_spmd(nc, in_maps, core_ids=list(range(len(in_maps))),
                               trace=trace)
    LAST_RESULTS = res
    outs = []
    for r in res.results:
        yt = r["y_t"]                                  # [NT, T*B] fp32
        outs.append(yt.reshape(NT, T, B).transpose(2, 1, 0))   # [B, T, NT]
    return np.concatenate(outs, axis=0).astype(np.float32)
